# revision 3
# baseline (speedup 1.0000x reference)
"""Expert-parallel MoE (top-2 of 16 experts) for 8 TRN2 NeuronCores.

Strategy (self-contained; shapes hardcoded for B=4,S=2048,H=1024,E=16,K=2,I=512):
  - Each core owns 2 experts (weights sharded over E); the full token set
    (bf16) is replicated into every core's HBM.
  - The router runs on each core's own 1/8 token shard (f32); logits are
    AllGather'd so every core sees the full routing.
  - Each core recomputes top-2 routing, builds its experts' compact token
    lists with the gpsimd sparse_gather op, gathers selected token rows with
    transposing dma_gather, runs gate_up -> glu -> down in bf16 (f32 psum),
    and scatters unscaled contribution rows (+down bias) into
    per-destination AllToAll slots via indirect DMA (OOB rows dropped).
  - One AllToAll exchanges contribution rows; each owner core then gathers
    its tokens' two contribution rows (indirect DMA), applies the softmax
    top-2 scores and writes its 1024x1024 f32 output shard.
"""
import sys
import types

import numpy as np
import ml_dtypes

# --- axon NTFF profile hook shim (lets run_bass_kernel_spmd(trace=True) work)
if "antenv.axon_hooks" not in sys.modules:
    try:
        import antenv

        _m = types.ModuleType("antenv.axon_hooks")
        _m._hook = None
        _m.set_axon_ntff_profile_hook = lambda h: setattr(_m, "_hook", h)
        _m.get_axon_ntff_profile_hook = lambda: _m._hook
        sys.modules["antenv.axon_hooks"] = _m
        antenv.axon_hooks = _m
        from trn_agent_boot.trn_boot import _ntff_profile_via_ctypes

        _m.set_axon_ntff_profile_hook(
            _ntff_profile_via_ctypes("/opt/axon/libaxon_pjrt.so")
        )
    except Exception:
        pass

import concourse.bass as bass
import concourse.mybir as mybir
import concourse.tile as tile
from concourse import bacc
from concourse.bass_utils import run_bass_kernel_spmd

bf16 = ml_dtypes.bfloat16
F32 = mybir.dt.float32
BF = mybir.dt.bfloat16
I16 = mybir.dt.int16
I32 = mybir.dt.int32
U8 = mybir.dt.uint8
U32 = mybir.dt.uint32
Alu = mybir.AluOpType
Act = mybir.ActivationFunctionType
AX = mybir.AxisListType

B, S, H = 4, 2048, 1024
T, E, K, INTER = 8192, 16, 2, 512
NC = 8
TLOC = T // NC
C_EXP = 1280            # compact capacity per expert
CW = C_EXP // 16        # 80 wrapped columns
C2 = 176                # slot capacity per (expert, owner core)
ROWS_PER_SRC = 2 * C2   # 352
TOT_ROWS = NC * ROWS_PER_SRC  # 2816
ALPHA, LIMIT = 1.702, 7.0
NEG = -1.0e30
CHUNKS = [512, 512, 256]

_CACHE = {}


def _build(trace_label=""):
    nc = bacc.Bacc("TRN2", target_bir_lowering=False, debug=False, num_devices=NC)

    xlocT = nc.declare_dram_parameter("xlocT", [H, TLOC], F32, isOutput=False)
    xfull = nc.declare_dram_parameter("xfull", [T, H], BF, isOutput=False)
    wr_p = nc.declare_dram_parameter("wr", [H, E], F32, isOutput=False)
    wgu_p = nc.declare_dram_parameter("wgu", [2, H, 2 * INTER], BF, isOutput=False)
    bgu_p = nc.declare_dram_parameter("bgu", [2, 2 * INTER], BF, isOutput=False)
    wd_p = nc.declare_dram_parameter("wd", [2, INTER, H], BF, isOutput=False)
    bd_p = nc.declare_dram_parameter("bd", [2, H], BF, isOutput=False)
    esel_p = nc.declare_dram_parameter("esel", [128, 2, E], F32, isOutput=False)
    iota1_p = nc.declare_dram_parameter("iota1", [128, 64], F32, isOutput=False)
    jio_p = nc.declare_dram_parameter("jio", [16, CW], F32, isOutput=False)
    iotaE_p = nc.declare_dram_parameter("iotaE", [128, E], F32, isOutput=False)
    dbound_p = nc.declare_dram_parameter("dbound", [16, NC], F32, isOutput=False)
    ones16_p = nc.declare_dram_parameter("ones16", [1, 16], F32, isOutput=False)
    o16bf_p = nc.declare_dram_parameter("o16bf", [16, 1], BF, isOutput=False)
    o128bf_p = nc.declare_dram_parameter("o128bf", [128, 1], BF, isOutput=False)
    ones1_p = nc.declare_dram_parameter("ones1", [1, 512], BF, isOutput=False)
    t128_p = nc.declare_dram_parameter("t128", [128, 128], BF, isOutput=False)
    out_p = nc.declare_dram_parameter("out", [TLOC, H], F32, isOutput=True)

    # internal DRAM (raw tensors: offset-0 APs for collectives/indirect DMA)
    lg_in = nc.dram_tensor("lg_in", [TLOC, E], F32)
    lg_out = nc.dram_tensor("lg_out", [T, E], F32)
    enc_d = nc.dram_tensor("enc_d", [2, T], F32)
    addr_d = nc.dram_tensor("addr_d", [2, C_EXP], I32)
    send_d = nc.dram_tensor("send_d", [TOT_ROWS, H], BF)
    recv_d = nc.dram_tensor("recv_d", [TOT_ROWS, H], BF)

    with tile.TileContext(nc) as tc:
        with (
            tc.tile_pool(name="w", bufs=1) as wp,
            tc.tile_pool(name="sb", bufs=1) as sb,
            tc.tile_pool(name="xg", bufs=2) as xgp,
            tc.tile_pool(name="act", bufs=2) as actp,
            tc.tile_pool(name="snd", bufs=3) as sndp,
            tc.tile_pool(name="rcv", bufs=2) as rcvp,
        ):
            # ---- weights / constants ----
            wgu_sb = wp.tile([128, 2, 8, 2 * INTER], BF)
            nc.sync.dma_start(wgu_sb[:], wgu_p.rearrange("e (k p) m -> p e k m", p=128))
            wd_sb = wp.tile([128, 2, 4, H], BF)
            nc.sync.dma_start(wd_sb[:], wd_p.rearrange("e (k p) m -> p e k m", p=128))
            bgu_sb = wp.tile([1, 2, 2 * INTER], BF)
            nc.sync.dma_start(bgu_sb[:], bgu_p[None, :, :])
            bd_sb = wp.tile([1, 2, H], BF)
            nc.sync.dma_start(bd_sb[:], bd_p[None, :, :])
            esel = wp.tile([128, 2, E], F32)
            nc.sync.dma_start(esel[:], esel_p[:])
            iota1 = wp.tile([128, 64], F32)
            nc.sync.dma_start(iota1[:], iota1_p[:])
            jio = wp.tile([16, CW], F32)
            nc.sync.dma_start(jio[:], jio_p[:])
            iotaE = wp.tile([128, E], F32)
            nc.sync.dma_start(iotaE[:], iotaE_p[:])
            dbound = wp.tile([16, NC], F32)
            nc.sync.dma_start(dbound[:], dbound_p[:])
            ones16 = wp.tile([1, 16], F32)
            nc.sync.dma_start(ones16[:], ones16_p[:])
            o16bf = wp.tile([16, 1], BF)
            nc.sync.dma_start(o16bf[:], o16bf_p[:])
            o128bf = wp.tile([128, 1], BF)
            nc.sync.dma_start(o128bf[:], o128bf_p[:])
            ones1 = wp.tile([1, 512], BF)
            nc.sync.dma_start(ones1[:], ones1_p[:])
            t128 = wp.tile([128, 128], BF)
            nc.sync.dma_start(t128[:], t128_p[:])

            # ================= router (own shard, f32) =================
            logits = sb.tile([128, 8, E], F32)
            wr_sb = sb.tile([128, 8, E], F32)
            nc.sync.dma_start(wr_sb[:], wr_p.rearrange("(k p) e -> p k e", p=128))
            with (
                tc.tile_pool(name="xl", bufs=3) as xlp,
                tc.tile_pool(name="psr", bufs=2, space="PSUM") as psr,
            ):
                for tt in range(8):
                    pl = psr.tile([128, E], F32, tag="pl")
                    for kk in range(8):
                        xlk = xlp.tile([128, 128], F32, tag="xl")
                        nc.sync.dma_start(
                            xlk[:],
                            xlocT[128 * kk:128 * (kk + 1),
                                  128 * tt:128 * (tt + 1)],
                        )
                        nc.tensor.matmul(pl[:], lhsT=xlk[:], rhs=wr_sb[:, kk, :],
                                         start=(kk == 0), stop=(kk == 7))
                    nc.vector.tensor_copy(logits[:, tt, :], pl[:])
            nc.sync.dma_start(lg_in.rearrange("(c p) e -> p c e", p=128), logits[:])

            # ================= AllGather logits =================
            nc.gpsimd.collective_compute(
                "AllGather", Alu.bypass,
                replica_groups=[list(range(NC))],
                ins=[lg_in.ap().opt()],
                outs=[lg_out.ap().opt()],
            )

            # ====== global top-2 -> my experts' enc lists (wrapped) ======
            enc_w = sb.tile([16, 2, T // 16], F32)
            with tc.tile_pool(name="g2", bufs=1) as g2:
                glog = g2.tile([128, 64, E], F32)
                nc.sync.dma_start(glog[:], lg_out.rearrange("(a p) e -> p a e", p=128))
                gm1 = g2.tile([128, 64, 1], F32)
                nc.vector.tensor_reduce(gm1[:], glog[:], axis=AX.X, op=Alu.max)
                geq1 = g2.tile([128, 64, E], F32)
                nc.vector.tensor_tensor(geq1[:], glog[:],
                                        gm1[:].to_broadcast([128, 64, E]),
                                        op=Alu.is_equal)
                gmsk = g2.tile([128, 64, E], F32)
                nc.vector.scalar_tensor_tensor(gmsk[:], in0=geq1[:], scalar=NEG,
                                               in1=glog[:], op0=Alu.mult,
                                               op1=Alu.add)
                gm2 = g2.tile([128, 64, 1], F32)
                nc.vector.tensor_reduce(gm2[:], gmsk[:], axis=AX.X, op=Alu.max)
                gboth = g2.tile([128, 64, E], F32)
                nc.vector.tensor_tensor(gboth[:], gmsk[:],
                                        gm2[:].to_broadcast([128, 64, E]),
                                        op=Alu.is_equal)
                nc.vector.tensor_add(gboth[:], gboth[:], geq1[:])
                for k in range(2):
                    sel = g2.tile([128, 64, E], F32, tag="sel")
                    nc.vector.tensor_tensor(
                        sel[:], gboth[:],
                        esel[:, k:k + 1, :].to_broadcast([128, 64, E]),
                        op=Alu.mult)
                    mek = g2.tile([128, 64], F32, tag="mek")
                    nc.vector.tensor_reduce(mek[:], sel[:], axis=AX.X, op=Alu.add)
                    enc = g2.tile([128, 64], F32, tag="encd")
                    nc.vector.tensor_tensor(enc[:], iota1[:], mek[:], op=Alu.mult)
                    nc.vector.tensor_scalar(enc[:], enc[:], 1.0, scalar2=None,
                                            op0=Alu.subtract)
                    nc.sync.dma_start(
                        enc_d[k].rearrange("(a p) -> p a", p=128), enc[:])
                    nc.sync.dma_start(
                        enc_w[:, k, :], enc_d[k].rearrange("(f p) -> p f", p=16))

            # ========== per-expert lists + scatter addresses ==========
            addr_sb = sb.tile([128, 2, C_EXP // 128], I32)
            lst16r = sb.tile([128, 2, CW], I16)
            with (
                tc.tile_pool(name="ix", bufs=1) as ix,
                tc.tile_pool(name="psx", bufs=1, space="PSUM") as psx,
            ):
                for k in range(2):
                    lst = ix.tile([16, CW], F32, tag="lst")
                    nfound = ix.tile([1, 1], U32, tag="nf")
                    nc.gpsimd.sparse_gather(lst[:], enc_w[:, k, :],
                                            num_found=nfound[:])
                    nff = ix.tile([1, 1], F32, tag="nff")
                    nc.vector.tensor_copy(nff[:], nfound[:])
                    nfb_ps = psx.tile([16, 1], F32, tag="nfb")
                    nc.tensor.matmul(nfb_ps[:], lhsT=ones16[:], rhs=nff[:],
                                     start=True, stop=True)
                    nfb = ix.tile([16, 1], F32, tag="nfbs")
                    nc.vector.tensor_copy(nfb[:], nfb_ps[:])
                    valid = ix.tile([16, CW], U8, tag="valid")
                    nc.vector.tensor_tensor(valid[:], jio[:],
                                            nfb[:].to_broadcast([16, CW]),
                                            op=Alu.is_lt)
                    tsafe = ix.tile([16, CW], F32, tag="tsafe")
                    nc.vector.memset(tsafe[:], 0.0)
                    nc.vector.copy_predicated(tsafe[:], valid[:], lst[:])

                    l16 = ix.tile([16, CW], I16, tag="l16")
                    nc.vector.tensor_copy(l16[:], tsafe[:])
                    for r in range(8):
                        nc.sync.dma_start(lst16r[16 * r:16 * (r + 1), k, :], l16[:])

                    # owner-core prefix starts
                    m16 = ix.tile([16, T // 16], BF, tag="m16")
                    nc.vector.tensor_scalar(m16[:], enc_w[:, k, :], 0.0,
                                            scalar2=None, op0=Alu.is_ge)
                    cs_ps = psx.tile([1, T // 16], F32, tag="csps")
                    nc.tensor.matmul(cs_ps[:], lhsT=o16bf[:], rhs=m16[:],
                                     start=True, stop=True)
                    cs = ix.tile([1, T // 16], F32, tag="cs")
                    nc.vector.tensor_copy(cs[:], cs_ps[:])
                    incl = ix.tile([1, T // 16], F32, tag="incl")
                    nc.vector.tensor_tensor_scan(incl[:], cs[:], cs[:], 0.0,
                                                 op0=Alu.add, op1=Alu.bypass)
                    starts = ix.tile([1, NC], F32, tag="starts")
                    nc.vector.memset(starts[:], 0.0)
                    nc.vector.tensor_copy(starts[0:1, 1:NC],
                                          incl[0:1, 63:449:64])
                    # telescoping lookup: start[d] = sum_{m<=d} delta[m],
                    # delta[0] = starts[0] = 0
                    delta = ix.tile([1, NC], F32, tag="delta")
                    nc.vector.memset(delta[0:1, 0:1], 0.0)
                    nc.vector.tensor_sub(delta[0:1, 1:NC], starts[0:1, 1:NC],
                                         starts[0:1, 0:NC - 1])
                    dl_ps = psx.tile([16, NC], F32, tag="dlps")
                    nc.tensor.matmul(dl_ps[:], lhsT=ones16[:], rhs=delta[:],
                                     start=True, stop=True)
                    delta_b = ix.tile([16, NC], F32, tag="deltab")
                    nc.vector.tensor_copy(delta_b[:], dl_ps[:])

                    oh = ix.tile([16, CW, NC], F32, tag="oh")
                    nc.vector.tensor_tensor(
                        oh[:],
                        tsafe[:, :, None].to_broadcast([16, CW, NC]),
                        dbound[:, None, :].to_broadcast([16, CW, NC]),
                        op=Alu.is_ge)
                    dsum = ix.tile([16, CW], F32, tag="dsum")
                    nc.vector.tensor_reduce(dsum[:], oh[:], axis=AX.X, op=Alu.add)
                    dj = ix.tile([16, CW], F32, tag="dj")
                    nc.vector.tensor_scalar(dj[:], dsum[:], 1.0, scalar2=None,
                                            op0=Alu.subtract)
                    nc.vector.tensor_tensor(
                        oh[:], oh[:],
                        delta_b[:, None, :].to_broadcast([16, CW, NC]),
                        op=Alu.mult)
                    stj = ix.tile([16, CW], F32, tag="stj")
                    nc.vector.tensor_reduce(stj[:], oh[:], axis=AX.X, op=Alu.add)
                    a1 = ix.tile([16, CW], F32, tag="a1")
                    nc.vector.scalar_tensor_tensor(
                        a1[:], in0=dj[:], scalar=float(ROWS_PER_SRC), in1=jio[:],
                        op0=Alu.mult, op1=Alu.add)
                    nc.vector.tensor_sub(a1[:], a1[:], stj[:])
                    if k:
                        nc.vector.tensor_scalar(a1[:], a1[:], float(C2),
                                                scalar2=None, op0=Alu.add)
                    abig = ix.tile([16, CW], F32, tag="abig")
                    nc.vector.memset(abig[:], 1.0e9)
                    nc.vector.copy_predicated(abig[:], valid[:], a1[:])
                    ai = ix.tile([16, CW], I32, tag="ai")
                    nc.vector.tensor_copy(ai[:], abig[:])
                    nc.sync.dma_start(
                        addr_d[k].rearrange("(f p) -> p f", p=16), ai[:])
                    nc.sync.dma_start(
                        addr_sb[:, k, :],
                        addr_d[k].rearrange("(c p) -> p c", p=128))

            # ================= expert MLPs =================
            with tc.tile_pool(name="psm", bufs=2, space="PSUM") as psm:
                for k in range(2):
                    off = 0
                    for TB in CHUNKS:
                        xgk = xgp.tile([128, 8, TB], BF, tag="xg")
                        nc.gpsimd.dma_gather(
                            xgk[:], xfull[:],
                            lst16r[:, k, off // 16:(off + TB) // 16],
                            num_idxs=TB, num_idxs_reg=TB, elem_size=H,
                            transpose=True)
                        act4 = actp.tile([128, 4, TB], BF, tag="act")
                        for i in range(4):
                            pg = psm.tile([128, TB], F32, tag="pg")
                            pu = psm.tile([128, TB], F32, tag="pu")
                            for kk in range(8):
                                nc.tensor.matmul(
                                    pg[:],
                                    lhsT=wgu_sb[:, k, kk, 128 * i:128 * (i + 1)],
                                    rhs=xgk[:, kk, :], start=(kk == 0), stop=False)
                            nc.tensor.matmul(
                                pg[:], lhsT=bgu_sb[0:1, k, 128 * i:128 * (i + 1)],
                                rhs=ones1[0:1, :TB], start=False, stop=True)
                            for kk in range(8):
                                nc.tensor.matmul(
                                    pu[:],
                                    lhsT=wgu_sb[:, k, kk, INTER + 128 * i:
                                                INTER + 128 * (i + 1)],
                                    rhs=xgk[:, kk, :], start=(kk == 0), stop=False)
                            nc.tensor.matmul(
                                pu[:],
                                lhsT=bgu_sb[0:1, k, INTER + 128 * i:
                                            INTER + 128 * (i + 1)],
                                rhs=ones1[0:1, :TB], start=False, stop=True)
                            g1 = sb.tile([128, 512], F32, tag="g1")
                            nc.vector.tensor_scalar_min(g1[:, :TB], pg[:], LIMIT)
                            sg = sb.tile([128, 512], F32, tag="sg")
                            nc.scalar.activation(sg[:, :TB], g1[:, :TB],
                                                 Act.Sigmoid, scale=ALPHA)
                            nc.vector.tensor_mul(g1[:, :TB], g1[:, :TB], sg[:, :TB])
                            u1 = sb.tile([128, 512], F32, tag="u1")
                            nc.vector.tensor_scalar(u1[:, :TB], pu[:], LIMIT,
                                                    scalar2=-LIMIT,
                                                    op0=Alu.min, op1=Alu.max)
                            nc.vector.scalar_tensor_tensor(
                                act4[:, i, :], in0=u1[:, :TB], scalar=1.0,
                                in1=g1[:, :TB], op0=Alu.add, op1=Alu.mult)
                        for tt in range(TB // 128):
                            pd = psm.tile([128, H], F32, tag="pd")
                            for hh in range(2):
                                for ki in range(4):
                                    nc.tensor.matmul(
                                        pd[:, 512 * hh:512 * (hh + 1)],
                                        lhsT=act4[:, ki, 128 * tt:128 * (tt + 1)],
                                        rhs=wd_sb[:, k, ki, 512 * hh:512 * (hh + 1)],
                                        start=(ki == 0), stop=False)
                                nc.tensor.matmul(
                                    pd[:, 512 * hh:512 * (hh + 1)],
                                    lhsT=ones1[0:1, :128],
                                    rhs=bd_sb[0:1, k, 512 * hh:512 * (hh + 1)],
                                    start=False, stop=True)
                            snd = sndp.tile([128, H], BF, tag="snd")
                            nc.vector.tensor_copy(snd[:], pd[:])
                            ci = (off + 128 * tt) // 128
                            nc.gpsimd.indirect_dma_start(
                                out=send_d[:],
                                out_offset=bass.IndirectOffsetOnAxis(
                                    ap=addr_sb[:, k, ci:ci + 1], axis=0),
                                in_=snd[:], in_offset=None,
                                bounds_check=TOT_ROWS - 1, oob_is_err=False)
                        off += TB

            # ================= AllToAll =================
            nc.gpsimd.collective_compute(
                "AllToAll", Alu.bypass,
                replica_groups=[list(range(NC))],
                ins=[send_d.ap().opt()],
                outs=[recv_d.ap().opt()],
            )

            # ================= receiver =================
            with tc.tile_pool(name="psv", bufs=1, space="PSUM") as psv:
                lm1 = sb.tile([128, 8, 1], F32)
                nc.vector.tensor_reduce(lm1[:], logits[:], axis=AX.X, op=Alu.max)
                leq1 = sb.tile([128, 8, E], F32)
                nc.vector.tensor_tensor(leq1[:], logits[:],
                                        lm1[:].to_broadcast([128, 8, E]),
                                        op=Alu.is_equal)
                lmsk = sb.tile([128, 8, E], F32)
                nc.vector.scalar_tensor_tensor(lmsk[:], in0=leq1[:], scalar=NEG,
                                               in1=logits[:], op0=Alu.mult,
                                               op1=Alu.add)
                lm2 = sb.tile([128, 8, 1], F32)
                nc.vector.tensor_reduce(lm2[:], lmsk[:], axis=AX.X, op=Alu.max)
                leq2 = sb.tile([128, 8, E], F32)
                nc.vector.tensor_tensor(leq2[:], lmsk[:],
                                        lm2[:].to_broadcast([128, 8, E]),
                                        op=Alu.is_equal)
                dif = sb.tile([128, 8], F32)
                nc.vector.tensor_sub(dif[:], lm1[:, :, 0], lm2[:, :, 0])
                p1 = sb.tile([128, 8], F32)
                nc.scalar.activation(p1[:], dif[:], Act.Sigmoid)
                p2 = sb.tile([128, 8], F32)
                nc.vector.tensor_scalar(p2[:], p1[:], -1.0, scalar2=1.0,
                                        op0=Alu.mult, op1=Alu.add)

                mtm = sb.tile([128, 128], BF)
                nc.vector.tensor_add(mtm[:], leq1[:].opt(), leq2[:].opt())
                cs2_ps = psv.tile([1, 128], F32, tag="cs2")
                nc.tensor.matmul(cs2_ps[:], lhsT=o128bf[:], rhs=mtm[:],
                                 start=True, stop=True)
                x0 = sb.tile([1, 128], F32, tag="x0a")
                nc.vector.tensor_copy(x0[:], cs2_ps[:])
                for sh in (16, 32, 64):
                    x1 = sb.tile([1, 128], F32, tag=f"xsh{sh}")
                    nc.vector.tensor_copy(x1[0:1, 0:sh], x0[0:1, 0:sh])
                    nc.vector.tensor_add(x1[0:1, sh:128], x0[0:1, sh:128],
                                         x0[0:1, 0:128 - sh])
                    x0 = x1
                excl = sb.tile([1, 128], BF)
                nc.vector.memset(excl[0:1, 0:16], 0.0)
                nc.vector.tensor_copy(excl[0:1, 16:128], x0[0:1, 0:112])
                pos2_ps = psv.tile([128, 128], F32, tag="pos2")
                nc.tensor.matmul(pos2_ps[:], lhsT=t128[:], rhs=mtm[:],
                                 start=True, stop=False)
                nc.tensor.matmul(pos2_ps[:], lhsT=ones1[0:1, :128], rhs=excl[:],
                                 start=False, stop=True)
                pos2 = sb.tile([128, 128], F32)
                nc.vector.tensor_copy(pos2[:], pos2_ps[:])

                addr_r = sb.tile([128, 2, 8], I32)
                for k, leq in ((0, leq1), (1, leq2)):
                    ekt = sb.tile([128, 8, E], F32, tag="ekt")
                    nc.vector.tensor_tensor(
                        ekt[:], leq[:],
                        iotaE[:, None, :].to_broadcast([128, 8, E]),
                        op=Alu.mult)
                    ek = sb.tile([128, 8], F32, tag="ek")
                    nc.vector.tensor_reduce(ek[:], ekt[:], axis=AX.X, op=Alu.add)
                    pk = sb.tile([128, 8, E], F32, tag="pk")
                    nc.vector.tensor_tensor(
                        pk[:], pos2[:].rearrange("p (c e) -> p c e", e=E),
                        leq[:], op=Alu.mult)
                    psk = sb.tile([128, 8], F32, tag="psk")
                    nc.vector.tensor_reduce(psk[:], pk[:], axis=AX.X, op=Alu.add)
                    af = sb.tile([128, 8], F32, tag="af")
                    nc.vector.scalar_tensor_tensor(
                        af[:], in0=ek[:], scalar=float(C2), in1=psk[:],
                        op0=Alu.mult, op1=Alu.add)
                    nc.vector.tensor_copy(addr_r[:, k, :], af[:])

                for tt in range(8):
                    r0 = rcvp.tile([128, H], BF, tag="r0")
                    nc.gpsimd.indirect_dma_start(
                        out=r0[:], out_offset=None, in_=recv_d[:],
                        in_offset=bass.IndirectOffsetOnAxis(
                            ap=addr_r[:, 0, tt:tt + 1], axis=0))
                    r1 = rcvp.tile([128, H], BF, tag="r1")
                    nc.gpsimd.indirect_dma_start(
                        out=r1[:], out_offset=None, in_=recv_d[:],
                        in_offset=bass.IndirectOffsetOnAxis(
                            ap=addr_r[:, 1, tt:tt + 1], axis=0))
                    o1 = rcvp.tile([128, H], F32, tag="o1")
                    nc.vector.tensor_tensor(
                        o1[:], r0[:], p1[:, tt:tt + 1].to_broadcast([128, H]),
                        op=Alu.mult)
                    o2 = rcvp.tile([128, H], F32, tag="o2")
                    nc.vector.tensor_tensor(
                        o2[:], r1[:], p2[:, tt:tt + 1].to_broadcast([128, H]),
                        op=Alu.mult)
                    nc.vector.tensor_add(o1[:], o1[:], o2[:])
                    nc.sync.dma_start(
                        out_p.rearrange("(c p) h -> p c h", p=128)[:, tt, :],
                        o1[:])

    nc.compile()
    return nc


def _consts():
    iota1 = (np.arange(64)[None, :] * 128 + np.arange(128)[:, None] + 1).astype(np.float32)
    jio = np.arange(C_EXP).reshape(CW, 16).T.astype(np.float32).copy()
    iotaE = np.broadcast_to(np.arange(E, dtype=np.float32), (128, E)).copy()
    dbound = np.broadcast_to((np.arange(NC) * TLOC).astype(np.float32),
                             (16, NC)).copy()
    return dict(
        iota1=iota1, jio=jio, iotaE=iotaE, dbound=dbound,
        ones16=np.ones((1, 16), np.float32),
        o16bf=np.ones((16, 1), bf16),
        o128bf=np.ones((128, 1), bf16),
        ones1=np.ones((1, 512), bf16),
        t128=(np.arange(128)[:, None] < np.arange(128)[None, :]).astype(bf16),
    )


def kernel(hidden_states, router_weight, gate_up_proj, gate_up_proj_bias,
           down_proj, down_proj_bias):
    x = np.ascontiguousarray(np.asarray(hidden_states, np.float32).reshape(T, H))
    wr = np.ascontiguousarray(np.asarray(router_weight, np.float32))
    wgu = np.asarray(gate_up_proj, np.float32)
    bgu = np.asarray(gate_up_proj_bias, np.float32)
    wd = np.asarray(down_proj, np.float32)
    bd = np.asarray(down_proj_bias, np.float32)

    x_bf = x.astype(bf16)
    wgu_perm = np.concatenate([wgu[:, :, 0::2], wgu[:, :, 1::2]], axis=2).astype(bf16)
    bgu_perm = np.concatenate([bgu[:, 0::2], bgu[:, 1::2]], axis=1).astype(bf16)
    wd_bf = wd.astype(bf16)
    bd_bf = bd.astype(bf16)
    consts = _consts()

    if "nc" not in _CACHE:
        _CACHE["nc"] = _build()
    nc = _CACHE["nc"]

    in_maps = []
    for c in range(NC):
        esel = np.zeros((128, 2, E), np.float32)
        esel[:, 0, 2 * c] = 1.0
        esel[:, 1, 2 * c + 1] = 1.0
        in_maps.append(dict(
            xlocT=np.ascontiguousarray(x[c * TLOC:(c + 1) * TLOC].T),
            xfull=x_bf,
            wr=wr,
            wgu=np.ascontiguousarray(wgu_perm[2 * c:2 * c + 2]),
            bgu=np.ascontiguousarray(bgu_perm[2 * c:2 * c + 2]),
            wd=np.ascontiguousarray(wd_bf[2 * c:2 * c + 2]),
            bd=np.ascontiguousarray(bd_bf[2 * c:2 * c + 2]),
            esel=esel,
            **consts,
        ))

    trace = bool(_CACHE.get("trace"))
    res = run_bass_kernel_spmd(nc, in_maps, core_ids=list(range(NC)),
                               trace=trace)
    if trace:
        _CACHE["last_result"] = res
    out = np.concatenate([r["out"] for r in res.results], axis=0)
    return out.reshape(B, S, H).astype(np.float32)


# revision 4
# speedup vs baseline: 1.0303x; 1.0303x over previous
"""Expert-parallel MoE (top-2 of 16 experts) for 8 TRN2 NeuronCores.

Strategy (self-contained; shapes hardcoded for B=4,S=2048,H=1024,E=16,K=2,I=512):
  - Each core owns 2 experts (weights sharded over E); the full token set
    (bf16) is replicated into every core's HBM.
  - The router runs on each core's own 1/8 token shard (f32); logits are
    AllGather'd so every core sees the full routing.
  - Each core recomputes top-2 routing, builds its experts' compact token
    lists with the gpsimd sparse_gather op, gathers selected token rows with
    transposing dma_gather, runs gate_up -> glu -> down in bf16 (f32 psum),
    and scatters unscaled contribution rows (+down bias) into
    per-destination AllToAll slots via indirect DMA (OOB rows dropped).
  - One AllToAll exchanges contribution rows; each owner core then gathers
    its tokens' two contribution rows (indirect DMA), applies the softmax
    top-2 scores and writes its 1024x1024 f32 output shard.
"""
import sys
import types

import numpy as np
import ml_dtypes

# --- axon NTFF profile hook shim (lets run_bass_kernel_spmd(trace=True) work)
if "antenv.axon_hooks" not in sys.modules:
    try:
        import antenv

        _m = types.ModuleType("antenv.axon_hooks")
        _m._hook = None
        _m.set_axon_ntff_profile_hook = lambda h: setattr(_m, "_hook", h)
        _m.get_axon_ntff_profile_hook = lambda: _m._hook
        sys.modules["antenv.axon_hooks"] = _m
        antenv.axon_hooks = _m
        from trn_agent_boot.trn_boot import _ntff_profile_via_ctypes

        _m.set_axon_ntff_profile_hook(
            _ntff_profile_via_ctypes("/opt/axon/libaxon_pjrt.so")
        )
    except Exception:
        pass

import concourse.bass as bass
import concourse.mybir as mybir
import concourse.tile as tile
from concourse import bacc
from concourse.bass_utils import run_bass_kernel_spmd

bf16 = ml_dtypes.bfloat16
F32 = mybir.dt.float32
BF = mybir.dt.bfloat16
I16 = mybir.dt.int16
I32 = mybir.dt.int32
U8 = mybir.dt.uint8
U32 = mybir.dt.uint32
Alu = mybir.AluOpType
Act = mybir.ActivationFunctionType
AX = mybir.AxisListType

B, S, H = 4, 2048, 1024
T, E, K, INTER = 8192, 16, 2, 512
NC = 8
TLOC = T // NC
C_EXP = 1280            # compact capacity per expert
CW = C_EXP // 16        # 80 wrapped columns
C2 = 176                # slot capacity per (expert, owner core)
ROWS_PER_SRC = 2 * C2   # 352
TOT_ROWS = NC * ROWS_PER_SRC  # 2816
ALPHA, LIMIT = 1.702, 7.0
NEG = -1.0e30
CHUNKS = [512, 512, 256]

_CACHE = {}


def _build(trace_label=""):
    nc = bacc.Bacc("TRN2", target_bir_lowering=False, debug=False, num_devices=NC)

    xlocT = nc.declare_dram_parameter("xlocT", [H, TLOC], F32, isOutput=False)
    xfull = nc.declare_dram_parameter("xfull", [T, H], BF, isOutput=False)
    wr_p = nc.declare_dram_parameter("wr", [H, E], F32, isOutput=False)
    wgu_p = nc.declare_dram_parameter("wgu", [2, H, 2 * INTER], BF, isOutput=False)
    bgu_p = nc.declare_dram_parameter("bgu", [128, 2, 8], F32, isOutput=False)
    wd_p = nc.declare_dram_parameter("wd", [2, INTER, H], BF, isOutput=False)
    bd_p = nc.declare_dram_parameter("bd", [128, 2, H], F32, isOutput=False)
    abase_p = nc.declare_dram_parameter("abase", [128, E], F32, isOutput=False)
    esel_p = nc.declare_dram_parameter("esel", [128, 2, E], F32, isOutput=False)
    iota1_p = nc.declare_dram_parameter("iota1", [128, 64], F32, isOutput=False)
    jio_p = nc.declare_dram_parameter("jio", [16, CW], F32, isOutput=False)
    iotaE_p = nc.declare_dram_parameter("iotaE", [128, E], F32, isOutput=False)
    dbound_p = nc.declare_dram_parameter("dbound", [16, NC], F32, isOutput=False)
    ones16_p = nc.declare_dram_parameter("ones16", [1, 16], F32, isOutput=False)
    o16bf_p = nc.declare_dram_parameter("o16bf", [16, 1], BF, isOutput=False)
    o128bf_p = nc.declare_dram_parameter("o128bf", [128, 1], BF, isOutput=False)
    ones1_p = nc.declare_dram_parameter("ones1", [1, 512], BF, isOutput=False)
    t128_p = nc.declare_dram_parameter("t128", [128, 128], BF, isOutput=False)
    out_p = nc.declare_dram_parameter("out", [TLOC, H], F32, isOutput=True)

    # internal DRAM (raw tensors: offset-0 APs for collectives/indirect DMA)
    lg_in = nc.dram_tensor("lg_in", [TLOC, E], F32)
    lg_out = nc.dram_tensor("lg_out", [T, E], F32)
    enc_d = nc.dram_tensor("enc_d", [2, T], F32)
    addr_d = nc.dram_tensor("addr_d", [2, C_EXP], I32)
    send_d = nc.dram_tensor("send_d", [TOT_ROWS, H], BF)
    recv_d = nc.dram_tensor("recv_d", [TOT_ROWS, H], BF)

    with tile.TileContext(nc) as tc:
        with (
            tc.tile_pool(name="w", bufs=1) as wp,
            tc.tile_pool(name="sb", bufs=1) as sb,
            tc.tile_pool(name="xg", bufs=2) as xgp,
            tc.tile_pool(name="act", bufs=2) as actp,
            tc.tile_pool(name="snd", bufs=3) as sndp,
            tc.tile_pool(name="rcv", bufs=2) as rcvp,
        ):
            # ---- weights / constants ----
            wgu_sb = wp.tile([128, 2, 8, 2 * INTER], BF)
            nc.sync.dma_start(wgu_sb[:], wgu_p.rearrange("e (k p) m -> p e k m", p=128))
            wd_sb = wp.tile([128, 2, 4, H], BF)
            nc.sync.dma_start(wd_sb[:], wd_p.rearrange("e (k p) m -> p e k m", p=128))
            bgu_sb = wp.tile([128, 2, 8], F32)
            nc.sync.dma_start(bgu_sb[:], bgu_p[:])
            bd_sb = wp.tile([128, 2, H], F32)
            nc.sync.dma_start(bd_sb[:], bd_p[:])
            abase = wp.tile([128, E], F32)
            nc.sync.dma_start(abase[:], abase_p[:])
            esel = wp.tile([128, 2, E], F32)
            nc.sync.dma_start(esel[:], esel_p[:])
            iota1 = wp.tile([128, 64], F32)
            nc.sync.dma_start(iota1[:], iota1_p[:])
            jio = wp.tile([16, CW], F32)
            nc.sync.dma_start(jio[:], jio_p[:])
            iotaE = wp.tile([128, E], F32)
            nc.sync.dma_start(iotaE[:], iotaE_p[:])
            dbound = wp.tile([16, NC], F32)
            nc.sync.dma_start(dbound[:], dbound_p[:])
            ones16 = wp.tile([1, 16], F32)
            nc.sync.dma_start(ones16[:], ones16_p[:])
            o16bf = wp.tile([16, 1], BF)
            nc.sync.dma_start(o16bf[:], o16bf_p[:])
            o128bf = wp.tile([128, 1], BF)
            nc.sync.dma_start(o128bf[:], o128bf_p[:])
            ones1 = wp.tile([1, 512], BF)
            nc.sync.dma_start(ones1[:], ones1_p[:])
            t128 = wp.tile([128, 128], BF)
            nc.sync.dma_start(t128[:], t128_p[:])

            # ================= router (own shard, f32) =================
            logits = sb.tile([128, 8, E], F32)
            wr_sb = sb.tile([128, 8, E], F32)
            nc.sync.dma_start(wr_sb[:], wr_p.rearrange("(k p) e -> p k e", p=128))
            with (
                tc.tile_pool(name="xl", bufs=1) as xlp,
                tc.tile_pool(name="psr", bufs=2, space="PSUM") as psr,
            ):
                xlT = xlp.tile([128, 8, TLOC], F32)
                nc.sync.dma_start(
                    xlT[:], xlocT.rearrange("(k p) t -> p k t", p=128))
                for tt in range(8):
                    pl = psr.tile([128, E], F32, tag="pl")
                    for kk in range(8):
                        nc.tensor.matmul(
                            pl[:],
                            lhsT=xlT[:, kk, 128 * tt:128 * (tt + 1)],
                            rhs=wr_sb[:, kk, :],
                            start=(kk == 0), stop=(kk == 7))
                    nc.vector.tensor_copy(logits[:, tt, :], pl[:])
            nc.sync.dma_start(lg_in.rearrange("(c p) e -> p c e", p=128), logits[:])

            # ================= AllGather logits =================
            nc.gpsimd.collective_compute(
                "AllGather", Alu.bypass,
                replica_groups=[list(range(NC))],
                ins=[lg_in.ap().opt()],
                outs=[lg_out.ap().opt()],
            )

            # ====== global top-2 -> my experts' enc lists (wrapped) ======
            enc_w = sb.tile([16, 2, T // 16], F32)
            with tc.tile_pool(name="g2", bufs=1) as g2:
                glog = g2.tile([128, 64, E], F32)
                nc.sync.dma_start(glog[:], lg_out.rearrange("(a p) e -> p a e", p=128))
                gm1 = g2.tile([128, 64, 1], F32)
                nc.vector.tensor_reduce(gm1[:], glog[:], axis=AX.X, op=Alu.max)
                geq1 = g2.tile([128, 64, E], F32)
                nc.vector.tensor_tensor(geq1[:], glog[:],
                                        gm1[:].to_broadcast([128, 64, E]),
                                        op=Alu.is_equal)
                gmsk = g2.tile([128, 64, E], F32)
                nc.vector.scalar_tensor_tensor(gmsk[:], in0=geq1[:], scalar=NEG,
                                               in1=glog[:], op0=Alu.mult,
                                               op1=Alu.add)
                gm2 = g2.tile([128, 64, 1], F32)
                nc.vector.tensor_reduce(gm2[:], gmsk[:], axis=AX.X, op=Alu.max)
                gboth = g2.tile([128, 64, E], F32)
                nc.vector.tensor_tensor(gboth[:], gmsk[:],
                                        gm2[:].to_broadcast([128, 64, E]),
                                        op=Alu.is_equal)
                nc.vector.tensor_add(gboth[:], gboth[:], geq1[:])
                for k in range(2):
                    sel = g2.tile([128, 64, E], F32, tag="sel")
                    nc.vector.tensor_tensor(
                        sel[:], gboth[:],
                        esel[:, k:k + 1, :].to_broadcast([128, 64, E]),
                        op=Alu.mult)
                    mek = g2.tile([128, 64], F32, tag="mek")
                    nc.vector.tensor_reduce(mek[:], sel[:], axis=AX.X, op=Alu.add)
                    enc = g2.tile([128, 64], F32, tag="encd")
                    nc.vector.tensor_tensor(enc[:], iota1[:], mek[:], op=Alu.mult)
                    nc.vector.tensor_scalar(enc[:], enc[:], 1.0, scalar2=None,
                                            op0=Alu.subtract)
                    nc.sync.dma_start(
                        enc_d[k].rearrange("(a p) -> p a", p=128), enc[:])
                    nc.sync.dma_start(
                        enc_w[:, k, :], enc_d[k].rearrange("(f p) -> p f", p=16))

            # ========== per-expert lists + scatter addresses ==========
            addr_sb = sb.tile([128, 2, C_EXP // 128], I32)
            lst16r = sb.tile([128, 2, CW], I16)
            with (
                tc.tile_pool(name="ix", bufs=1) as ix,
                tc.tile_pool(name="psx", bufs=1, space="PSUM") as psx,
            ):
                for k in range(2):
                    lst = ix.tile([16, CW], F32, tag="lst")
                    nfound = ix.tile([1, 1], U32, tag="nf")
                    nc.gpsimd.sparse_gather(lst[:], enc_w[:, k, :],
                                            num_found=nfound[:])
                    nff = ix.tile([1, 1], F32, tag="nff")
                    nc.vector.tensor_copy(nff[:], nfound[:])
                    nfb_ps = psx.tile([16, 1], F32, tag="nfb")
                    nc.tensor.matmul(nfb_ps[:], lhsT=ones16[:], rhs=nff[:],
                                     start=True, stop=True)
                    nfb = ix.tile([16, 1], F32, tag="nfbs")
                    nc.vector.tensor_copy(nfb[:], nfb_ps[:])
                    valid = ix.tile([16, CW], U8, tag="valid")
                    nc.vector.tensor_tensor(valid[:], jio[:],
                                            nfb[:].to_broadcast([16, CW]),
                                            op=Alu.is_lt)
                    tsafe = ix.tile([16, CW], F32, tag="tsafe")
                    nc.vector.memset(tsafe[:], 0.0)
                    nc.vector.copy_predicated(tsafe[:], valid[:], lst[:])

                    l16 = ix.tile([16, CW], I16, tag="l16")
                    nc.vector.tensor_copy(l16[:], tsafe[:])
                    for r in range(8):
                        nc.sync.dma_start(lst16r[16 * r:16 * (r + 1), k, :], l16[:])

                    # owner-core prefix starts
                    m16 = ix.tile([16, T // 16], BF, tag="m16")
                    nc.vector.tensor_scalar(m16[:], enc_w[:, k, :], 0.0,
                                            scalar2=None, op0=Alu.is_ge)
                    cs_ps = psx.tile([1, T // 16], F32, tag="csps")
                    nc.tensor.matmul(cs_ps[:], lhsT=o16bf[:], rhs=m16[:],
                                     start=True, stop=True)
                    cs = ix.tile([1, T // 16], F32, tag="cs")
                    nc.vector.tensor_copy(cs[:], cs_ps[:])
                    incl = ix.tile([1, T // 16], F32, tag="incl")
                    nc.vector.tensor_tensor_scan(incl[:], cs[:], cs[:], 0.0,
                                                 op0=Alu.add, op1=Alu.bypass)
                    starts = ix.tile([1, NC], F32, tag="starts")
                    nc.vector.memset(starts[:], 0.0)
                    nc.vector.tensor_copy(starts[0:1, 1:NC],
                                          incl[0:1, 63:449:64])
                    # telescoping lookup: start[d] = sum_{m<=d} delta[m],
                    # delta[0] = starts[0] = 0
                    delta = ix.tile([1, NC], F32, tag="delta")
                    nc.vector.memset(delta[0:1, 0:1], 0.0)
                    nc.vector.tensor_sub(delta[0:1, 1:NC], starts[0:1, 1:NC],
                                         starts[0:1, 0:NC - 1])
                    dl_ps = psx.tile([16, NC], F32, tag="dlps")
                    nc.tensor.matmul(dl_ps[:], lhsT=ones16[:], rhs=delta[:],
                                     start=True, stop=True)
                    delta_b = ix.tile([16, NC], F32, tag="deltab")
                    nc.vector.tensor_copy(delta_b[:], dl_ps[:])

                    oh = ix.tile([16, CW, NC], F32, tag="oh")
                    nc.vector.tensor_tensor(
                        oh[:],
                        tsafe[:, :, None].to_broadcast([16, CW, NC]),
                        dbound[:, None, :].to_broadcast([16, CW, NC]),
                        op=Alu.is_ge)
                    dsum = ix.tile([16, CW], F32, tag="dsum")
                    nc.vector.tensor_reduce(dsum[:], oh[:], axis=AX.X, op=Alu.add)
                    dj = ix.tile([16, CW], F32, tag="dj")
                    nc.vector.tensor_scalar(dj[:], dsum[:], 1.0, scalar2=None,
                                            op0=Alu.subtract)
                    nc.vector.tensor_tensor(
                        oh[:], oh[:],
                        delta_b[:, None, :].to_broadcast([16, CW, NC]),
                        op=Alu.mult)
                    stj = ix.tile([16, CW], F32, tag="stj")
                    nc.vector.tensor_reduce(stj[:], oh[:], axis=AX.X, op=Alu.add)
                    a1 = ix.tile([16, CW], F32, tag="a1")
                    nc.vector.scalar_tensor_tensor(
                        a1[:], in0=dj[:], scalar=float(C2), in1=jio[:],
                        op0=Alu.mult, op1=Alu.add)
                    nc.vector.tensor_sub(a1[:], a1[:], stj[:])
                    if k:
                        nc.vector.tensor_scalar(a1[:], a1[:], float(NC * C2),
                                                scalar2=None, op0=Alu.add)
                    abig = ix.tile([16, CW], F32, tag="abig")
                    nc.vector.memset(abig[:], 1.0e9)
                    nc.vector.copy_predicated(abig[:], valid[:], a1[:])
                    ai = ix.tile([16, CW], I32, tag="ai")
                    nc.vector.tensor_copy(ai[:], abig[:])
                    nc.sync.dma_start(
                        addr_d[k].rearrange("(f p) -> p f", p=16), ai[:])
                    nc.sync.dma_start(
                        addr_sb[:, k, :],
                        addr_d[k].rearrange("(c p) -> p c", p=128))

            # ================= expert MLPs =================
            HB = NC * C2  # 1408 rows per expert half-buffer
            with tc.tile_pool(name="psm", bufs=2, space="PSUM") as psm:
                for k in range(2):
                    off = 0
                    for TB in CHUNKS:
                        xgk = xgp.tile([128, 8, TB], BF, tag="xg")
                        nc.gpsimd.dma_gather(
                            xgk[:], xfull[:],
                            lst16r[:, k, off // 16:(off + TB) // 16],
                            num_idxs=TB, num_idxs_reg=TB, elem_size=H,
                            transpose=True)
                        act4 = actp.tile([128, 4, TB], BF, tag="act")
                        for i in range(4):
                            pg = psm.tile([128, TB], F32, tag="pg")
                            pu = psm.tile([128, TB], F32, tag="pu")
                            for kk in range(8):
                                nc.tensor.matmul(
                                    pg[:],
                                    lhsT=wgu_sb[:, k, kk, 128 * i:128 * (i + 1)],
                                    rhs=xgk[:, kk, :], start=(kk == 0),
                                    stop=(kk == 7))
                            for kk in range(8):
                                nc.tensor.matmul(
                                    pu[:],
                                    lhsT=wgu_sb[:, k, kk, INTER + 128 * i:
                                                INTER + 128 * (i + 1)],
                                    rhs=xgk[:, kk, :], start=(kk == 0),
                                    stop=(kk == 7))
                            # g1 = min(g + bias, 7)
                            g1 = sb.tile([128, 512], F32, tag="g1")
                            nc.vector.tensor_scalar(
                                g1[:, :TB], pg[:], bgu_sb[:, k, 2 * i:2 * i + 1],
                                scalar2=LIMIT, op0=Alu.add, op1=Alu.min)
                            sg = sb.tile([128, 512], F32, tag="sg")
                            nc.scalar.activation(sg[:, :TB], g1[:, :TB],
                                                 Act.Sigmoid, scale=ALPHA)
                            nc.vector.tensor_mul(g1[:, :TB], g1[:, :TB], sg[:, :TB])
                            # u1 = max(min(u + bias, 7), -7)
                            u1 = sb.tile([128, 512], F32, tag="u1")
                            nc.vector.tensor_scalar(
                                u1[:, :TB], pu[:], bgu_sb[:, k, 2 * i + 1:2 * i + 2],
                                scalar2=LIMIT, op0=Alu.add, op1=Alu.min)
                            nc.vector.tensor_scalar_max(u1[:, :TB], u1[:, :TB],
                                                        -LIMIT)
                            nc.vector.scalar_tensor_tensor(
                                act4[:, i, :], in0=u1[:, :TB], scalar=1.0,
                                in1=g1[:, :TB], op0=Alu.add, op1=Alu.mult)
                        for tt in range(TB // 128):
                            pd = psm.tile([128, H], F32, tag="pd")
                            for hh in range(2):
                                for ki in range(4):
                                    nc.tensor.matmul(
                                        pd[:, 512 * hh:512 * (hh + 1)],
                                        lhsT=act4[:, ki, 128 * tt:128 * (tt + 1)],
                                        rhs=wd_sb[:, k, ki, 512 * hh:512 * (hh + 1)],
                                        start=(ki == 0), stop=(ki == 3))
                            # + down bias (pre-broadcast), cast to bf16
                            snd = sndp.tile([128, H], BF, tag="snd")
                            nc.vector.tensor_add(snd[:], pd[:], bd_sb[:, k, :])
                            ci = (off + 128 * tt) // 128
                            nc.gpsimd.indirect_dma_start(
                                out=send_d[:],
                                out_offset=bass.IndirectOffsetOnAxis(
                                    ap=addr_sb[:, k, ci:ci + 1], axis=0),
                                in_=snd[:], in_offset=None,
                                bounds_check=TOT_ROWS - 1, oob_is_err=False)
                        off += TB
                    # per-expert AllToAll on this expert's half buffer:
                    # overlaps the next expert's compute
                    nc.gpsimd.collective_compute(
                        "AllToAll", Alu.bypass,
                        replica_groups=[list(range(NC))],
                        ins=[send_d.ap()[k * HB:(k + 1) * HB, :].opt()],
                        outs=[recv_d.ap()[k * HB:(k + 1) * HB, :].opt()],
                    )

            # ================= receiver =================
            with tc.tile_pool(name="psv", bufs=1, space="PSUM") as psv:
                lm1 = sb.tile([128, 8, 1], F32)
                nc.vector.tensor_reduce(lm1[:], logits[:], axis=AX.X, op=Alu.max)
                leq1 = sb.tile([128, 8, E], F32)
                nc.vector.tensor_tensor(leq1[:], logits[:],
                                        lm1[:].to_broadcast([128, 8, E]),
                                        op=Alu.is_equal)
                lmsk = sb.tile([128, 8, E], F32)
                nc.vector.scalar_tensor_tensor(lmsk[:], in0=leq1[:], scalar=NEG,
                                               in1=logits[:], op0=Alu.mult,
                                               op1=Alu.add)
                lm2 = sb.tile([128, 8, 1], F32)
                nc.vector.tensor_reduce(lm2[:], lmsk[:], axis=AX.X, op=Alu.max)
                leq2 = sb.tile([128, 8, E], F32)
                nc.vector.tensor_tensor(leq2[:], lmsk[:],
                                        lm2[:].to_broadcast([128, 8, E]),
                                        op=Alu.is_equal)
                dif = sb.tile([128, 8], F32)
                nc.vector.tensor_sub(dif[:], lm1[:, :, 0], lm2[:, :, 0])
                p1 = sb.tile([128, 8], F32)
                nc.scalar.activation(p1[:], dif[:], Act.Sigmoid)
                p2 = sb.tile([128, 8], F32)
                nc.vector.tensor_scalar(p2[:], p1[:], -1.0, scalar2=1.0,
                                        op0=Alu.mult, op1=Alu.add)

                mtm = sb.tile([128, 128], BF)
                nc.vector.tensor_add(mtm[:], leq1[:].opt(), leq2[:].opt())
                cs2_ps = psv.tile([1, 128], F32, tag="cs2")
                nc.tensor.matmul(cs2_ps[:], lhsT=o128bf[:], rhs=mtm[:],
                                 start=True, stop=True)
                x0 = sb.tile([1, 128], F32, tag="x0a")
                nc.vector.tensor_copy(x0[:], cs2_ps[:])
                for sh in (16, 32, 64):
                    x1 = sb.tile([1, 128], F32, tag=f"xsh{sh}")
                    nc.vector.tensor_copy(x1[0:1, 0:sh], x0[0:1, 0:sh])
                    nc.vector.tensor_add(x1[0:1, sh:128], x0[0:1, sh:128],
                                         x0[0:1, 0:128 - sh])
                    x0 = x1
                excl = sb.tile([1, 128], BF)
                nc.vector.memset(excl[0:1, 0:16], 0.0)
                nc.vector.tensor_copy(excl[0:1, 16:128], x0[0:1, 0:112])
                pos2_ps = psv.tile([128, 128], F32, tag="pos2")
                nc.tensor.matmul(pos2_ps[:], lhsT=t128[:], rhs=mtm[:],
                                 start=True, stop=False)
                nc.tensor.matmul(pos2_ps[:], lhsT=ones1[0:1, :128], rhs=excl[:],
                                 start=False, stop=True)
                pos2 = sb.tile([128, 128], F32)
                nc.vector.tensor_copy(pos2[:], pos2_ps[:])

                addr_r = sb.tile([128, 2, 8], I32)
                for k, leq in ((0, leq1), (1, leq2)):
                    ekt = sb.tile([128, 8, E], F32, tag="ekt")
                    nc.vector.tensor_tensor(
                        ekt[:], leq[:],
                        abase[:, None, :].to_broadcast([128, 8, E]),
                        op=Alu.mult)
                    ek = sb.tile([128, 8], F32, tag="ek")
                    nc.vector.tensor_reduce(ek[:], ekt[:], axis=AX.X, op=Alu.add)
                    pk = sb.tile([128, 8, E], F32, tag="pk")
                    nc.vector.tensor_tensor(
                        pk[:], pos2[:].rearrange("p (c e) -> p c e", e=E),
                        leq[:], op=Alu.mult)
                    psk = sb.tile([128, 8], F32, tag="psk")
                    nc.vector.tensor_reduce(psk[:], pk[:], axis=AX.X, op=Alu.add)
                    af = sb.tile([128, 8], F32, tag="af")
                    nc.vector.tensor_add(af[:], ek[:], psk[:])
                    nc.vector.tensor_copy(addr_r[:, k, :], af[:])

                for tt in range(8):
                    r0 = rcvp.tile([128, H], BF, tag="r0")
                    nc.gpsimd.indirect_dma_start(
                        out=r0[:], out_offset=None, in_=recv_d[:],
                        in_offset=bass.IndirectOffsetOnAxis(
                            ap=addr_r[:, 0, tt:tt + 1], axis=0))
                    r1 = rcvp.tile([128, H], BF, tag="r1")
                    nc.gpsimd.indirect_dma_start(
                        out=r1[:], out_offset=None, in_=recv_d[:],
                        in_offset=bass.IndirectOffsetOnAxis(
                            ap=addr_r[:, 1, tt:tt + 1], axis=0))
                    o1 = rcvp.tile([128, H], F32, tag="o1")
                    nc.vector.tensor_tensor(
                        o1[:], r0[:], p1[:, tt:tt + 1].to_broadcast([128, H]),
                        op=Alu.mult)
                    o2 = rcvp.tile([128, H], F32, tag="o2")
                    nc.vector.tensor_tensor(
                        o2[:], r1[:], p2[:, tt:tt + 1].to_broadcast([128, H]),
                        op=Alu.mult)
                    nc.vector.tensor_add(o1[:], o1[:], o2[:])
                    nc.sync.dma_start(
                        out_p.rearrange("(c p) h -> p c h", p=128)[:, tt, :],
                        o1[:])

    nc.compile()
    return nc


def _consts():
    iota1 = (np.arange(64)[None, :] * 128 + np.arange(128)[:, None] + 1).astype(np.float32)
    jio = np.arange(C_EXP).reshape(CW, 16).T.astype(np.float32).copy()
    iotaE = np.broadcast_to(np.arange(E, dtype=np.float32), (128, E)).copy()
    dbound = np.broadcast_to((np.arange(NC) * TLOC).astype(np.float32),
                             (16, NC)).copy()
    return dict(
        iota1=iota1, jio=jio, iotaE=iotaE, dbound=dbound,
        ones16=np.ones((1, 16), np.float32),
        o16bf=np.ones((16, 1), bf16),
        o128bf=np.ones((128, 1), bf16),
        ones1=np.ones((1, 512), bf16),
        t128=(np.arange(128)[:, None] < np.arange(128)[None, :]).astype(bf16),
        abase=np.broadcast_to(
            (np.arange(E) % 2) * (NC * C2) + (np.arange(E) // 2) * C2,
            (128, E)).astype(np.float32),
    )


def kernel(hidden_states, router_weight, gate_up_proj, gate_up_proj_bias,
           down_proj, down_proj_bias):
    x = np.ascontiguousarray(np.asarray(hidden_states, np.float32).reshape(T, H))
    wr = np.ascontiguousarray(np.asarray(router_weight, np.float32))
    wgu = np.asarray(gate_up_proj, np.float32)
    bgu = np.asarray(gate_up_proj_bias, np.float32)
    wd = np.asarray(down_proj, np.float32)
    bd = np.asarray(down_proj_bias, np.float32)

    x_bf = x.astype(bf16)
    wgu_perm = np.concatenate([wgu[:, :, 0::2], wgu[:, :, 1::2]], axis=2).astype(bf16)
    # bgu_t[p, e_loc, 2i+g] layouts: per-partition bias for inter chunk i
    # (gate at even slots, up at odd slots)
    wd_bf = wd.astype(bf16)
    # per-partition gate/up biases: bgu_t[p, e, 2i] = gate bias(i*128+p),
    # bgu_t[p, e, 2i+1] = up bias
    gate_b = bgu[:, 0::2]  # [E, 512]
    up_b = bgu[:, 1::2]
    bgu_t = np.zeros((128, E, 8), np.float32)
    for i in range(4):
        bgu_t[:, :, 2 * i] = gate_b[:, 128 * i:128 * (i + 1)].T
        bgu_t[:, :, 2 * i + 1] = up_b[:, 128 * i:128 * (i + 1)].T
    bd_bc = np.broadcast_to(bd[None, :, :], (128, E, H)).astype(np.float32)
    consts = _consts()

    if "nc" not in _CACHE:
        _CACHE["nc"] = _build()
    nc = _CACHE["nc"]

    in_maps = []
    for c in range(NC):
        esel = np.zeros((128, 2, E), np.float32)
        esel[:, 0, 2 * c] = 1.0
        esel[:, 1, 2 * c + 1] = 1.0
        in_maps.append(dict(
            xlocT=np.ascontiguousarray(x[c * TLOC:(c + 1) * TLOC].T),
            xfull=x_bf,
            wr=wr,
            wgu=np.ascontiguousarray(wgu_perm[2 * c:2 * c + 2]),
            bgu=np.ascontiguousarray(bgu_t[:, 2 * c:2 * c + 2]),
            wd=np.ascontiguousarray(wd_bf[2 * c:2 * c + 2]),
            bd=np.ascontiguousarray(bd_bc[:, 2 * c:2 * c + 2]),
            esel=esel,
            **consts,
        ))

    trace = bool(_CACHE.get("trace"))
    res = run_bass_kernel_spmd(nc, in_maps, core_ids=list(range(NC)),
                               trace=trace)
    if trace:
        _CACHE["last_result"] = res
    out = np.concatenate([r["out"] for r in res.results], axis=0)
    return out.reshape(B, S, H).astype(np.float32)
